# revision 12
# baseline (speedup 1.0000x reference)
"""Trainium2 Bass kernel for nn_DeeperGCN_LineGraph (8-core SPMD).

Sharding: line-graph NODES (= base-graph edges, E_G=131072) are split into 8
contiguous ranges of 16384. Each core owns the line-graph edges whose *dst*
falls in its range, sorted by dst, so all segment reductions are core-local.
Per layer, the gathered quantity q = x * nb is AllGather'ed across cores; each
core then gathers q[src] for its local lg-edges with indirect DMA.

Key algorithmic transforms (validated vs reference in numpy to ~6e-7 rel):
 - softmax aggregation computed without the segment_max pass (values are tiny;
   exp never overflows; empty segments give num=den=0 -> agg=0 via
   rec = 1/max(den, 0.5) since any nonempty den >= 1).
 - m = relu(y)+eps folded: ex = Exp(t*m) via ACT with scale=t; num integrand
   ex*m includes the eps term exactly.
 - encoder h0 = [x_g[src]|x_g[dst]|ea_g|x_lg|1] @ Wcomb with
   Wcomb = [W_enc@W_msg[:H]; W_enc@W_msg[H:2H]; W_msg[2H:]; b_enc@(W1+W2)+b_msg]
 - eval-mode BN folded to per-feature scale/shift (per-partition in the
   column-major layout).
 - segment sums via one-hot matmuls: edges (partitions) x [exm|ex] against a
   per-chunk one-hot built with tensor_scalar(is_equal) vs an iota row; chunks
   are host-padded so each chunk belongs to exactly one 128-segment window.
 - graph pooling composed through dst: pooled[b] = sum_e h[e]*[gid(e)==b],
   gid(e) = batch_idx[dst_g[e]]; one matmul chain into a [H, B] PSUM tile.

State layout on device is column-major hT [H=128 partitions, segs free], which
makes BN+ReLU a single ACT op (per-partition scale/bias), and the layer matmul
h_newT = W.T @ (xT + naT) a full-rate N=512 matmul with W stationary.
"""
import os
import numpy as np

H = 128
WIN = 128          # segments per window
CH = 128           # edge slots per chunk
NCORES = 8
EPS_MSG = 1e-7
BN_EPS = 1e-5
B_GRAPHS = 256
DTYPE = np.float32


# --------------------------------------------------------------------------
# host-side preprocessing
# --------------------------------------------------------------------------
def _preprocess(inputs, n_cores=NCORES, b_graphs=B_GRAPHS):
    x_g = np.asarray(inputs['x_g'], DTYPE)
    edge_attr_g = np.asarray(inputs['edge_attr_g'], DTYPE)
    x_lg = np.asarray(inputs['x_lg'], DTYPE)
    edge_dist_basis = np.asarray(inputs['edge_dist_basis'], DTYPE)
    edge_attr_lg = np.asarray(inputs['edge_attr_lg'], DTYPE)
    edge_index_g = np.asarray(inputs['edge_index_g'])
    edge_index_lg = np.asarray(inputs['edge_index_lg'])
    batch_idx = np.asarray(inputs['batch_idx'])
    W_enc = np.asarray(inputs['W_enc'], DTYPE); b_enc = np.asarray(inputs['b_enc'], DTYPE)
    W_msg = np.asarray(inputs['W_msg'], DTYPE); b_msg = np.asarray(inputs['b_msg'], DTYPE)
    W_nb = np.asarray(inputs['W_nb'], DTYPE); b_nb = np.asarray(inputs['b_nb'], DTYPE)
    W_eb = np.asarray(inputs['W_eb'], DTYPE); b_eb = np.asarray(inputs['b_eb'], DTYPE)
    W_gcn = np.asarray(inputs['W_gcn'], DTYPE); b_gcn = np.asarray(inputs['b_gcn'], DTYPE)
    t_vals = np.asarray(inputs['t_vals'], DTYPE)
    gamma = np.asarray(inputs['gamma'], DTYPE); beta = np.asarray(inputs['beta'], DTYPE)

    N_G, node_dim = x_g.shape
    E_G = edge_attr_g.shape[0]
    E_LG = edge_attr_lg.shape[0]
    edge_dim = edge_attr_g.shape[1]
    nb_dim = edge_dist_basis.shape[1]
    eb_dim = edge_attr_lg.shape[1]
    L = W_gcn.shape[0]
    EPC = E_G // n_cores
    NW = EPC // WIN

    inv = np.float32(1.0 / np.sqrt(np.float32(1.0 + BN_EPS)))

    # folded weights
    W1 = W_msg[0:H]; W2 = W_msg[H:2*H]
    W3 = W_msg[2*H:2*H+edge_dim]; W4 = W_msg[2*H+edge_dim:]
    Wcomb = np.concatenate([W_enc @ W1, W_enc @ W2, W3, W4,
                            (b_enc @ W1 + b_enc @ W2 + b_msg)[None, :]],
                           axis=0).astype(DTYPE)          # [2*nd+ed+nbd+1, H]
    Wnb5 = np.concatenate([W_nb, b_nb[None, :]], axis=0).astype(DTYPE)
    Web5 = np.concatenate([W_eb, b_eb[None, :]], axis=0).astype(DTYPE)
    gammaS = (inv * gamma).T.copy().astype(DTYPE)          # [H, L]
    betaS = beta.T.copy().astype(DTYPE)                    # [H, L]
    bgT = b_gcn.T.copy().astype(DTYPE)                     # [H, L]

    src_g, dst_g = edge_index_g[0], edge_index_g[1]
    src_lg, dst_lg = edge_index_lg[0], edge_index_lg[1]

    # ---- per-core lg-edge partition, sorted by dst, chunked per window ----
    owner = dst_lg // EPC
    per_core = []
    cpw_max = 1
    for c in range(n_cores):
        sel = np.where(owner == c)[0]
        order = np.argsort(dst_lg[sel], kind='stable')
        e_ids = sel[order]
        dloc = (dst_lg[e_ids] - c * EPC).astype(np.int64)
        woe = dloc // WIN
        # edges per window
        cnt = np.bincount(woe, minlength=NW)
        cpw_max = max(cpw_max, int(np.max((cnt + CH - 1) // CH)) if len(cnt) else 1)
        per_core.append((e_ids, dloc, woe, cnt))

    CPW = cpw_max
    NCH = NW * CPW
    NSLOT = NCH * CH

    cores = []
    for c in range(n_cores):
        e_ids, dloc, woe, cnt = per_core[c]
        # slot arrays
        gidx = np.zeros((NCH, CH), np.int32)       # gather row (global) per slot
        relf = np.full((NCH, CH), -1.0, DTYPE)     # one-hot rel idx, -1 = pad
        slot_eid = np.full((NCH, CH), -1, np.int64)
        pos = np.concatenate([[0], np.cumsum(cnt)])
        for w in range(NW):
            i0, i1 = pos[w], pos[w + 1]
            n = i1 - i0
            nch_w = max(1, (n + CH - 1) // CH)
            for k in range(nch_w):
                a, b_ = i0 + k * CH, min(i0 + (k + 1) * CH, i1)
                m = b_ - a
                kk = w * CPW + k
                if m > 0:
                    gidx[kk, :m] = src_lg[e_ids[a:b_]]
                    relf[kk, :m] = (dloc[a:b_] - w * WIN).astype(DTYPE)
                    slot_eid[kk, :m] = e_ids[a:b_]
        # ebfT: [eb_dim+1, NSLOT] in slot order (chunk k, partition p) -> col k*CH+p
        flat_eid = slot_eid.reshape(-1)
        ebf = np.zeros((NSLOT, eb_dim + 1), DTYPE)
        valid = flat_eid >= 0
        ebf[valid, :eb_dim] = edge_attr_lg[flat_eid[valid]]
        ebf[valid, eb_dim] = 1.0
        ebfT = ebf.T.copy()
        # gidx/relf device layout: [CH(partition), NCH] with idx[p, k] = slot(k, p)
        gidxT = gidx.T.copy()                      # [CH, NCH]
        relfT = relf.T.copy()                      # [CH, NCH]

        lo = c * EPC
        sl = slice(lo, lo + EPC)
        encT = np.concatenate([x_g[src_g[sl]], x_g[dst_g[sl]],
                               edge_attr_g[sl], x_lg[sl],
                               np.ones((EPC, 1), DTYPE)], axis=1).T.copy()
        nbfT = np.concatenate([edge_dist_basis[sl],
                               np.ones((EPC, 1), DTYPE)], axis=1).T.copy()
        # pooling graph ids: gidf[p, w] = batch_idx[dst_g[lo + w*WIN + p]]
        gidf = batch_idx[dst_g[sl]].astype(DTYPE).reshape(NW, WIN).T.copy()
        cores.append(dict(encT=encT, nbfT=nbfT, ebfT=ebfT,
                          gidxT=gidxT, relfT=relfT, gidf=gidf))

    counts = np.bincount(batch_idx, minlength=b_graphs).astype(DTYPE)

    cfg = dict(N_G=N_G, E_G=E_G, E_LG=E_LG, EPC=EPC, NW=NW, CPW=CPW,
               NCH=NCH, NSLOT=NSLOT, L=L, B=b_graphs, n_cores=n_cores,
               enc_k=Wcomb.shape[0], nb_k=Wnb5.shape[0], eb_k=Web5.shape[0])
    host = dict(Wcomb=Wcomb, Wnb5=Wnb5, Web5=Web5, gammaS=gammaS,
                betaS=betaS, bgT=bgT, Wg=W_gcn, t_vals=t_vals, counts=counts)
    return cfg, host, cores


# --------------------------------------------------------------------------
# device program
# --------------------------------------------------------------------------
def _build_program(cfg, host):
    import concourse.bass as bass
    import concourse.mybir as mybir
    import concourse.tile as tile
    from concourse import bacc
    from concourse.masks import make_identity

    f32 = mybir.dt.float32
    f32r = mybir.dt.float32r
    bf16 = mybir.dt.bfloat16
    i32 = mybir.dt.int32
    A = mybir.AluOpType
    AF = mybir.ActivationFunctionType

    EPC, NW, CPW, NCH = cfg['EPC'], cfg['NW'], cfg['CPW'], cfg['NCH']
    NSLOT, L, B, E_G = cfg['NSLOT'], cfg['L'], cfg['B'], cfg['E_G']
    n_cores = cfg['n_cores']
    enc_k, nb_k, eb_k = cfg['enc_k'], cfg['nb_k'], cfg['eb_k']
    GW = 4 if NW % 4 == 0 else (2 if NW % 2 == 0 else 1)   # windows per group
    NG = NW // GW                                          # groups
    SC = GW * WIN                                          # segs per group
    t_vals = [float(x) for x in host['t_vals']]

    nc = bacc.Bacc("TRN2", target_bir_lowering=False, debug=False,
                   num_devices=n_cores)

    # ---- I/O ----
    encT_d = nc.dram_tensor("encT", [enc_k, EPC], f32, kind="ExternalInput")
    nbfT_d = nc.dram_tensor("nbfT", [nb_k, EPC], f32, kind="ExternalInput")
    ebfT_d = nc.dram_tensor("ebfT", [eb_k, NSLOT], f32, kind="ExternalInput")
    gidx_d = nc.dram_tensor("gidxT", [CH, NCH], i32, kind="ExternalInput")
    relf_d = nc.dram_tensor("relfT", [CH, NCH], f32, kind="ExternalInput")
    gidf_d = nc.dram_tensor("gidf", [WIN, NW], f32, kind="ExternalInput")
    Wcomb_d = nc.dram_tensor("Wcomb", [enc_k, H], f32, kind="ExternalInput")
    Wnb5_d = nc.dram_tensor("Wnb5", [nb_k, H], f32, kind="ExternalInput")
    Web5_d = nc.dram_tensor("Web5", [eb_k, H], f32, kind="ExternalInput")
    Wg_d = nc.dram_tensor("Wg", [L, H, H], f32, kind="ExternalInput")
    gammaS_d = nc.dram_tensor("gammaS", [H, L], f32, kind="ExternalInput")
    betaS_d = nc.dram_tensor("betaS", [H, L], f32, kind="ExternalInput")
    bgT_d = nc.dram_tensor("bgT", [H, L], f32, kind="ExternalInput")
    pool_d = nc.dram_tensor("poolT", [H, B], f32, kind="ExternalOutput")

    q_full = [nc.dram_tensor(f"q_full{i}", [E_G, H], f32, kind="Internal",
                             addr_space="Shared") for i in range(2)]

    from contextlib import ExitStack
    with tile.TileContext(nc) as tc:
        ctx = ExitStack()
        with ctx:
            state = ctx.enter_context(tc.tile_pool(name="state", bufs=1))
            consts = ctx.enter_context(tc.tile_pool(name="consts", bufs=1))
            sb = ctx.enter_context(tc.tile_pool(name="sb", bufs=2))
            sb2 = ctx.enter_context(tc.tile_pool(name="sb2", bufs=2))
            ps_nd = ctx.enter_context(tc.tile_pool(name="ps_nd", bufs=2, space="PSUM"))
            ps_a = ctx.enter_context(tc.tile_pool(name="ps_a", bufs=2, space="PSUM"))
            ps_b = ctx.enter_context(tc.tile_pool(name="ps_b", bufs=2, space="PSUM"))
            dram = ctx.enter_context(tc.tile_pool(name="dram", bufs=2, space="DRAM"))

            # ---- persistent state ----
            hT = state.tile([H, EPC], f32, tag="hT")

            # ---- constants to SBUF ----
            ident = consts.tile([128, 128], f32, tag="ident")
            make_identity(nc, ident[:])
            iota_i = consts.tile([128, 128], i32, tag="iota_i")
            nc.gpsimd.iota(iota_i[:], pattern=[[1, 128]], base=0,
                           channel_multiplier=0)
            iota_f = consts.tile([128, 128], f32, tag="iota_f")
            nc.vector.tensor_copy(iota_f[:], iota_i[:])
            iota_bi = consts.tile([128, B], i32, tag="iota_bi")
            nc.gpsimd.iota(iota_bi[:], pattern=[[1, B]], base=0,
                           channel_multiplier=0)
            iota_bf = consts.tile([128, B], f32, tag="iota_bf")
            nc.vector.tensor_copy(iota_bf[:], iota_bi[:])

            Wcomb_s = consts.tile([enc_k, H], f32, tag="Wcomb")
            nc.sync.dma_start(out=Wcomb_s[:], in_=Wcomb_d.ap())
            Wnb5_s = consts.tile([nb_k, H], f32, tag="Wnb5")
            nc.sync.dma_start(out=Wnb5_s[:], in_=Wnb5_d.ap())
            Web5_s = consts.tile([eb_k, H], f32, tag="Web5")
            nc.sync.dma_start(out=Web5_s[:], in_=Web5_d.ap())
            Wg_s = consts.tile([H, L, H], f32, tag="Wg")
            nc.sync.dma_start(out=Wg_s[:],
                              in_=Wg_d.ap().rearrange("l k h -> k l h"))
            gamma_s = consts.tile([H, L], f32, tag="gamma")
            nc.sync.dma_start(out=gamma_s[:], in_=gammaS_d.ap())
            beta_s = consts.tile([H, L], f32, tag="beta")
            nc.sync.dma_start(out=beta_s[:], in_=betaS_d.ap())
            bg_s = consts.tile([H, L], f32, tag="bg")
            nc.sync.dma_start(out=bg_s[:], in_=bgT_d.ap())
            gidx_s = consts.tile([CH, NCH], i32, tag="gidx")
            nc.sync.dma_start(out=gidx_s[:], in_=gidx_d.ap())
            relf_s = consts.tile([CH, NCH], f32, tag="relf")
            nc.sync.dma_start(out=relf_s[:], in_=relf_d.ap())
            gidf_s = consts.tile([WIN, NW], f32, tag="gidf")
            nc.sync.dma_start(out=gidf_s[:], in_=gidf_d.ap())

            def r32(ap):
                # plain fp32 for now: walrus requires f32r matmul inputs to be
                # produced as f32r by the upstream instruction (rounded), not
                # bitcast. Producer-side f32r typing is a later optimization.
                return ap

            # ---- encoder: hT = Wcomb.T @ encT ----
            for g in range(NG):
                sl = slice(g * SC, (g + 1) * SC)
                enc_t = sb.tile([enc_k, SC], f32, tag="enc_t")
                nc.sync.dma_start(out=enc_t[:], in_=encT_d.ap()[:, sl])
                hp = ps_a.tile([H, SC], f32, tag="mm")
                nc.tensor.matmul(hp[:], r32(Wcomb_s[:]), r32(enc_t[:]),
                                 start=True, stop=True)
                nc.scalar.copy(out=hT[:, sl], in_=hp[:])

            # ---- layers ----
            for l in range(L):
                t = t_vals[l]
                qf = q_full[l % 2]
                q_in = dram.tile([EPC, H], f32, tag="q_in")
                # phase 1: xT, q rows -> q_in
                for g in range(NG):
                    sl = slice(g * SC, (g + 1) * SC)
                    if l == 0:
                        xg = hT[:, sl]
                    else:
                        xg_t = sb.tile([H, SC], f32, tag="xg")
                        nc.scalar.activation(out=xg_t[:], in_=hT[:, sl],
                                             func=AF.Relu,
                                             bias=beta_s[:, l - 1:l],
                                             scale=gamma_s[:, l - 1:l])
                        xg = xg_t[:]
                    nbf_t = sb.tile([nb_k, SC], f32, tag="nbf_t")
                    nc.sync.dma_start(out=nbf_t[:], in_=nbfT_d.ap()[:, sl])
                    nbp = ps_a.tile([H, SC], f32, tag="mm")
                    nc.tensor.matmul(nbp[:], r32(Wnb5_s[:]), r32(nbf_t[:]),
                                     start=True, stop=True)
                    qt = sb.tile([H, SC], f32, tag="qt")
                    nc.vector.tensor_tensor(out=qt[:], in0=xg,
                                            in1=nbp[:], op=A.mult)
                    qrp = ps_b.tile([128, GW, 128], f32, tag="tp")
                    for j in range(GW):
                        nc.tensor.transpose(out=qrp[:, j, :],
                                            in_=qt[:, j * 128:(j + 1) * 128],
                                            identity=ident[:])
                    qr = sb.tile([128, GW, 128], f32, tag="qr")
                    nc.scalar.copy(out=qr[:], in_=qrp[:])
                    nc.sync.dma_start(
                        out=q_in[g * SC:(g + 1) * SC].rearrange(
                            "(j p) h -> p j h", p=128),
                        in_=qr[:])
                # phase 2: AllGather
                nc.gpsimd.collective_compute(
                    "AllGather", mybir.AluOpType.bypass,
                    replica_groups=[list(range(n_cores))],
                    ins=[q_in[:]], outs=[qf.ap()])
                # phase 3+4: per window group
                for g in range(NG):
                    sl = slice(g * SC, (g + 1) * SC)
                    k0 = g * GW * CPW          # first chunk of group
                    nchg = GW * CPW            # chunks in group
                    # gather all rows for the group in one indirect DMA
                    qs = sb2.tile([CH, nchg, H], f32, tag="qs")
                    nc.gpsimd.indirect_dma_start(
                        out=qs[:], out_offset=None,
                        in_=qf.ap(),
                        in_offset=bass.IndirectOffsetOnAxis(
                            ap=gidx_s[:, k0:k0 + nchg], axis=0))
                    nd = ps_nd.tile([128, GW, 2 * H], f32, tag="nd")
                    exmex = sb2.tile([CH, nchg, 2 * H], f32, tag="exmex")
                    # sub-batches of 4 chunks for PSUM-coupled elementwise
                    SB = 4 if nchg % 4 == 0 else (2 if nchg % 2 == 0 else 1)
                    for s0 in range(0, nchg, SB):
                        ssl = slice(s0, s0 + SB)
                        ebf_t = sb.tile([eb_k, SB * CH], f32, tag="ebf_t")
                        nc.sync.dma_start(
                            out=ebf_t[:],
                            in_=ebfT_d.ap()[:, (k0 + s0) * CH:(k0 + s0 + SB) * CH])
                        ebp = ps_a.tile([128, SB, H], f32, tag="mm")
                        for j in range(SB):
                            nc.tensor.matmul(
                                ebp[:, j, :],
                                r32(ebf_t[:, j * CH:(j + 1) * CH]),
                                r32(Web5_s[:]), start=True, stop=True)
                        # y = qs*eb ; m = max(y,0)+eps  (in place in qs)
                        nc.vector.tensor_tensor(out=qs[:, ssl, :],
                                                in0=qs[:, ssl, :],
                                                in1=ebp[:], op=A.mult)
                        nc.vector.tensor_scalar(qs[:, ssl, :], qs[:, ssl, :],
                                                0.0, EPS_MSG, A.max, A.add)
                        # ex = exp(t*m) ; exm = ex*m
                        nc.scalar.activation(out=exmex[:, ssl, H:2 * H],
                                             in_=qs[:, ssl, :],
                                             func=AF.Exp, scale=t)
                        nc.vector.tensor_tensor(out=exmex[:, ssl, 0:H],
                                                in0=exmex[:, ssl, H:2 * H],
                                                in1=qs[:, ssl, :], op=A.mult)
                    # one-hots + segsum matmuls
                    for j in range(nchg):
                        k = k0 + j
                        w = j // CPW           # window within group
                        p = j % CPW
                        oh = sb.tile([CH, 128], f32, tag="oh")
                        nc.vector.tensor_scalar(oh[:], iota_f[:],
                                                relf_s[:, k:k + 1], None,
                                                A.is_equal)
                        nc.tensor.matmul(nd[:, w, :], r32(oh[:]),
                                         r32(exmex[:, j, :]),
                                         start=(p == 0), stop=(p == CPW - 1))
                    # phase 4: agg, update
                    rec = sb.tile([128, GW, H], f32, tag="rec")
                    nc.vector.tensor_scalar(rec[:], nd[:, :, H:2 * H],
                                            0.5, None, A.max)
                    nc.vector.reciprocal(rec[:], rec[:])
                    na = sb.tile([128, GW, H], f32, tag="na")
                    nc.vector.tensor_tensor(out=na[:], in0=nd[:, :, 0:H],
                                            in1=rec[:], op=A.mult)
                    natp = ps_b.tile([128, GW, 128], f32, tag="tp")
                    for j in range(GW):
                        nc.tensor.transpose(out=natp[:, j, :],
                                            in_=na[:, j, :],
                                            identity=ident[:])
                    if l == 0:
                        xg = hT[:, sl]
                    else:
                        xg_t = sb.tile([H, SC], f32, tag="xg4")
                        nc.scalar.activation(out=xg_t[:], in_=hT[:, sl],
                                             func=AF.Relu,
                                             bias=beta_s[:, l - 1:l],
                                             scale=gamma_s[:, l - 1:l])
                        xg = xg_t[:]
                    xna = sb.tile([H, SC], f32, tag="xna")
                    nc.vector.tensor_tensor(out=xna[:], in0=natp[:].rearrange(
                        "p j h -> p (j h)"), in1=xg, op=A.add)
                    hnp = ps_a.tile([H, SC], f32, tag="mm")
                    nc.tensor.matmul(hnp[:], r32(Wg_s[:, l, :]), r32(xna[:]),
                                     start=True, stop=True)
                    if l == 0:
                        nc.vector.tensor_scalar(hT[:, sl], hnp[:],
                                                bg_s[:, l:l + 1], None, A.add)
                    else:
                        hs = sb.tile([H, SC], f32, tag="hs")
                        nc.vector.tensor_tensor(out=hs[:], in0=hnp[:],
                                                in1=hT[:, sl], op=A.add)
                        nc.vector.tensor_scalar(hT[:, sl], hs[:],
                                                bg_s[:, l:l + 1], None, A.add)

            # ---- final bn + pooling ----
            poolp = ps_nd.tile([H, B], f32, tag="nd")
            for g in range(NG):
                sl = slice(g * SC, (g + 1) * SC)
                hf = sb.tile([H, SC], f32, tag="hf")
                nc.scalar.activation(out=hf[:], in_=hT[:, sl],
                                     func=AF.Identity,
                                     bias=beta_s[:, L - 1:L],
                                     scale=gamma_s[:, L - 1:L])
                hfrp = ps_b.tile([128, GW, 128], f32, tag="tp")
                for j in range(GW):
                    nc.tensor.transpose(out=hfrp[:, j, :],
                                        in_=hf[:, j * 128:(j + 1) * 128],
                                        identity=ident[:])
                hfr = sb.tile([128, GW, 128], f32, tag="hfr")
                nc.scalar.copy(out=hfr[:], in_=hfrp[:])
                for j in range(GW):
                    w = g * GW + j
                    ohg = sb.tile([WIN, B], f32, tag="ohg")
                    nc.vector.tensor_scalar(ohg[:], iota_bf[:WIN, :],
                                            gidf_s[:, w:w + 1], None,
                                            A.is_equal)
                    nc.tensor.matmul(poolp[:], r32(hfr[:, j, :]), r32(ohg[:]),
                                     start=(w == 0), stop=(w == NW - 1))
            pool_s = sb.tile([H, B], f32, tag="pool_s")
            nc.vector.tensor_copy(out=pool_s[:], in_=poolp[:])
            nc.sync.dma_start(out=pool_d.ap(), in_=pool_s[:])

    nc.compile()
    return nc


# --------------------------------------------------------------------------
# entry point
# --------------------------------------------------------------------------
def _enable_axon_trace():
    """Install the NTFF profile hook shim (antenv.axon_hooks is absent on
    slim agent images) so run_bass_kernel_spmd(trace=True) works under axon."""
    import sys
    import types
    try:
        import antenv.axon_hooks  # noqa: F401
    except ImportError:
        mod = types.ModuleType("antenv.axon_hooks")
        holder = [None]
        mod.set_axon_ntff_profile_hook = lambda h: holder.__setitem__(0, h)
        mod.get_axon_ntff_profile_hook = lambda: holder[0]
        sys.modules["antenv.axon_hooks"] = mod
        import antenv
        antenv.axon_hooks = mod
    from antenv.axon_hooks import (get_axon_ntff_profile_hook,
                                   set_axon_ntff_profile_hook)
    if get_axon_ntff_profile_hook() is None:
        from trn_agent_boot.trn_boot import _ntff_profile_via_ctypes
        set_axon_ntff_profile_hook(
            _ntff_profile_via_ctypes('/opt/axon/libaxon_pjrt.so'))
    from concourse import bass_utils
    bass_utils.upload_artifacts = lambda tmpdir: str(tmpdir)
    # the axon profiler can only attach after the PJRT client has executed
    # something in this process
    import jax
    import jax.numpy as jnp
    np.asarray(jax.jit(lambda a: a + 1)(jnp.zeros((8,), jnp.float32)))


def kernel(**inputs):
    from concourse import bass_utils

    W_pred = np.asarray(inputs['W_pred'], DTYPE)
    b_pred = np.asarray(inputs['b_pred'], DTYPE)

    cfg, host, cores = _preprocess(inputs)
    nc = _build_program(cfg, host)

    in_maps = []
    for c in range(cfg['n_cores']):
        cc = cores[c]
        in_maps.append({
            'encT': cc['encT'], 'nbfT': cc['nbfT'], 'ebfT': cc['ebfT'],
            'gidxT': cc['gidxT'], 'relfT': cc['relfT'], 'gidf': cc['gidf'],
            'Wcomb': host['Wcomb'], 'Wnb5': host['Wnb5'], 'Web5': host['Web5'],
            'Wg': host['Wg'], 'gammaS': host['gammaS'], 'betaS': host['betaS'],
            'bgT': host['bgT'],
        })

    trace = os.environ.get('KERNEL_TRACE', '0') == '1'
    kw = {}
    if trace:
        _enable_axon_trace()
        import tempfile
        kw['tmpdir'] = tempfile.mkdtemp(prefix='ktrace_')
    res = bass_utils.run_bass_kernel_spmd(
        nc, in_maps, core_ids=list(range(cfg['n_cores'])), trace=trace, **kw)
    if trace:
        kernel.last_exec_time_ns = res.exec_time_ns
        kernel.last_profile = res.profile_json
        kernel.last_trace_dir = kw.get('tmpdir')

    pooledT = np.zeros((H, cfg['B']), DTYPE)
    for c in range(cfg['n_cores']):
        pooledT += res.results[c]['poolT']
    gsum = pooledT.T
    h_graph = gsum / np.maximum(host['counts'], 1.0)[:, None]
    return (h_graph @ W_pred + b_pred).astype(DTYPE)
